# revision 1
# baseline (speedup 1.0000x reference)
"""ArcFace loss kernel for 8 Trainium2 NeuronCores (class-parallel sharding).

Strategy (per sharding hint): shard the class dimension of the weight matrix
across the 8 cores (12.5k classes each, padded to 12800). Each core streams
its weight shard once from HBM (the memory roofline), computes the per-shard
plain-softmax partial sums S_b = sum_c exp(s*cos(b,c) - s), and one 2KB
AllReduce combines them. The ArcFace margin only touches one (b, target)
element per row, so the margin correction + final loss are computed
replicated on every core from the (host-gathered) target weight rows.

The weight shard is passed host-side pre-transposed as (D, C_shard) so the
contraction dim D lands on SBUF partitions (the layout the TensorEngine
requires); this is a data-layout choice for the chosen sharding, all
arithmetic of the reference loss happens on device.
"""

import sys

sys.path.insert(0, "/opt/trn_rl_repo")

import math

import numpy as np

import concourse.bass as bass
import concourse.bacc as bacc
import concourse.mybir as mybir
import concourse.tile as tile
from concourse.masks import make_identity
from concourse.bass_utils import run_bass_kernel_spmd

# ---- problem constants (hardcoded per the task contract) ----
B = 512  # batch
D = 512  # embedding dim
C = 100000  # num classes
NCORES = 8
CS = 12800  # padded classes per core (8*12800 = 102400 >= 100000)
CB = 512  # class block (free dim per psum bank)
NBLK = CS // CB  # 25
NB = B // 128  # 4 batch chunks
ND = D // 128  # 4 contraction chunks
NSUP = 5  # super-blocks for the resident bf16 weight load
NPAIR = (NBLK + 1) // 2  # 13 exp groups (pairs of class blocks)
USE_PBCAST = False  # GPSIMD partition_broadcast measured slower than the PE replication matmul

SCALE = 64.0
MARGIN = 0.5
COS_M = math.cos(MARGIN)
SIN_M = math.sin(MARGIN)
THRESHOLD = math.cos(math.pi - MARGIN)
MARGIN_MULT = math.sin(math.pi - MARGIN) * MARGIN
MSHIFT = 14.0  # fixed logsumexp shift; exp args land in the well-fitted spline region
# (still overflow-safe: max arg = 64-14 = 50 -> e^50 ~ 5e21, far below f32 max)

F32 = mybir.dt.float32
F32R = mybir.dt.float32r
BF16 = mybir.dt.bfloat16
Act = mybir.ActivationFunctionType
Alu = mybir.AluOpType
AX = mybir.AxisListType


def _ttr(nc, out, in0, in1, accum):
    """accum = sum_free(in0 * in1).

    (InstTensorTensorReduce hangs the runtime in this environment, so use a
    plain mult + free-axis reduce pair instead.)
    """
    nc.vector.tensor_tensor(out=out, in0=in0, in1=in1, op=Alu.mult)
    nc.vector.tensor_reduce(out=accum, in_=out, axis=AX.X, op=Alu.add)


def _emit_exp(nc, mp, item, Scols, mneg_b):
    g, b, pidx, width = item
    esc = mp.tile([128, 2 * CB], F32, tag="esc", bufs=2, name="esc")
    nc.scalar.activation(
        esc[:, 0:width],
        g[:, 0:width],
        Act.Exp,
        scale=SCALE,
        bias=mneg_b,
        accum_out=Scols[b][:, pidx : pidx + 1],
    )


def _patch_act_tables():
    """Restrict the ACT-table chooser to the combined ln+exp set.

    The default chooser pairs Ln with `natural_log` and Exp with
    `exp_and_others`, reloading tables (~2.7us each) every block.  All
    activations used here (Ln, Exp, Copy) live in
    `natural_log_exp_and_others`, so force that single set.
    """
    from concourse import hw_specs

    if getattr(bacc, "_act_tables_patched", False):
        return
    orig = bacc.get_activation_tables

    def prefer_combined(arch):
        t = orig(arch)
        if "natural_log_exp_and_others" not in t:
            return t
        out = {}
        strip = {Act.Exp, Act.Ln}
        for k, v in t.items():
            if k == "natural_log_exp_and_others":
                out[k] = v
            else:
                out[k] = {f for f in v if f not in strip}
        return out

    bacc.get_activation_tables = prefer_combined
    bacc._act_tables_patched = True


def build_kernel(stage=3, repeat=1, ablate=()):
    _patch_act_tables()
    nc = bacc.Bacc("TRN2", target_bir_lowering=False, debug=False, num_devices=NCORES)

    e_h = nc.declare_dram_parameter("e", [B, D], F32, isOutput=False)
    wtgt_h = nc.declare_dram_parameter("wtgt", [B, D], F32, isOutput=False)
    wt_h = nc.declare_dram_parameter("wt", [D, CS], F32, isOutput=False)
    out_h = nc.declare_dram_parameter("out", [1, 1], F32, isOutput=True)

    with tile.TileContext(nc) as tc:
        for _ in range(repeat):
            _body(nc, tc, e_h, wtgt_h, wt_h, out_h, stage, set(ablate))
    nc.compile()
    return nc


def _body(nc, tc, e_h, wtgt_h, wt_h, out_h, stage=3, ablate=frozenset()):
    with tc.tile_pool(name="const", bufs=1) as cp:
        idn = cp.tile([128, 128], F32, tag="idn")
        make_identity(nc, idn)
        ones_bf = cp.tile([128, 1], BF16, tag="ones_bf")
        nc.vector.memset(ones_bf, 1.0)
        ones_k1 = cp.tile([1, 128], BF16, tag="ones_k1")
        nc.vector.memset(ones_k1, 1.0)
        ones_f = cp.tile([128, 1], F32, tag="ones_f")
        nc.vector.memset(ones_f, 1.0)
        eps_b = cp.tile([128, 1], F32, tag="eps_b")
        nc.vector.memset(eps_b, 1e-24)
        mneg_b = cp.tile([128, 1], F32, tag="mneg_b")
        nc.vector.memset(mneg_b, -MSHIFT)

        # persistent state
        eT_bf = [cp.tile([128, B], BF16, tag=f"eT{j}", name=f"eT{j}") for j in range(ND)]
        e_n = [cp.tile([128, D], F32, tag=f"en{k}", name=f"en{k}") for k in range(NB)]
        Scols = [
            cp.tile([128, NPAIR], F32, tag=f"scols{b}", name=f"scols{b}")
            for b in range(NB)
        ]
        corr = cp.tile([128, NB], F32, tag="corr")
        tterm = cp.tile([128, NB], F32, tag="tterm")

        # ---------------- Phase A: embeddings + target rows (replicated) ---
        with (
            tc.tile_pool(name="prep", bufs=1) as pp,
            tc.tile_pool(name="prep_ps", bufs=2, space="PSUM") as ppp,
        ):
            e_sb = pp.tile([128, NB * D], F32, tag="eload")
            nc.sync.dma_start(
                out=e_sb.rearrange("p (k d) -> p k d", k=NB),
                in_=e_h[:].rearrange("(k p) d -> p k d", p=128),
            )
            wt_sb = pp.tile([128, NB * D], F32, tag="wtload")
            nc.sync.dma_start(
                out=wt_sb.rearrange("p (k d) -> p k d", k=NB),
                in_=wtgt_h[:].rearrange("(k p) d -> p k d", p=128),
            )

            junk = pp.tile([128, D], F32, tag="junk", bufs=2)
            ne2 = pp.tile([128, NB], F32, tag="ne2")
            nt2 = pp.tile([128, NB], F32, tag="nt2")
            dotr = pp.tile([128, NB], F32, tag="dotr")
            for k in range(NB):
                ek = e_sb[:, k * D : (k + 1) * D]
                _ttr(nc, junk, ek, ek, ne2[:, k : k + 1])
            # inv_ne = exp(-0.5*ln(ne2 + eps^2))  (rsqrt via the ln/exp set)
            lnb = pp.tile([128, NB], F32, tag="lnb")
            nc.scalar.activation(lnb, ne2, Act.Ln, bias=eps_b)
            inv_ne = pp.tile([128, NB], F32, tag="inv_ne")
            nc.scalar.activation(inv_ne, lnb, Act.Exp, scale=-0.5)
            for k in range(NB):
                nc.vector.tensor_scalar(
                    out=e_n[k],
                    in0=e_sb[:, k * D : (k + 1) * D],
                    scalar1=inv_ne[:, k : k + 1],
                    scalar2=None,
                    op0=Alu.mult,
                )
            # eT (transposed, bf16) for the matmul stationary side
            for k in range(NB):
                for j in range(ND):
                    tp = ppp.tile([128, 128], F32, tag="tp", bufs=2)
                    nc.tensor.transpose(tp, e_n[k][:, j * 128 : (j + 1) * 128], idn)
                    nc.vector.tensor_copy(
                        out=eT_bf[j][:, k * 128 : (k + 1) * 128], in_=tp
                    )

            # target rows: norms + dot with e_n
            for k in range(NB):
                wk = wt_sb[:, k * D : (k + 1) * D]
                _ttr(nc, junk, wk, wk, nt2[:, k : k + 1])
            lnt = pp.tile([128, NB], F32, tag="lnt")
            nc.scalar.activation(lnt, nt2, Act.Ln, bias=eps_b)
            inv_nt = pp.tile([128, NB], F32, tag="inv_nt")
            nc.scalar.activation(inv_nt, lnt, Act.Exp, scale=-0.5)
            for k in range(NB):
                _ttr(
                    nc,
                    junk,
                    e_n[k],
                    wt_sb[:, k * D : (k + 1) * D],
                    dotr[:, k : k + 1],
                )
            cos_t = pp.tile([128, NB], F32, tag="cos_t")
            nc.vector.tensor_tensor(out=cos_t, in0=dotr, in1=inv_nt, op=Alu.mult)

            # phi = cos*cos_m - sin*sin_m ; easy-margin=False branch
            c2 = pp.tile([128, NB], F32, tag="c2")
            nc.vector.tensor_tensor(out=c2, in0=cos_t, in1=cos_t, op=Alu.mult)
            om = pp.tile([128, NB], F32, tag="om")
            nc.vector.tensor_scalar(
                out=om, in0=c2, scalar1=-1.0, scalar2=1.0, op0=Alu.mult, op1=Alu.add
            )
            nc.vector.tensor_scalar(
                out=om, in0=om, scalar1=1e-30, scalar2=None, op0=Alu.max
            )
            lnz = pp.tile([128, NB], F32, tag="lnz")
            nc.scalar.activation(lnz, om, Act.Ln)
            sine = pp.tile([128, NB], F32, tag="sine")
            nc.scalar.activation(sine, lnz, Act.Exp, scale=0.5)
            sinm = pp.tile([128, NB], F32, tag="sinm")
            nc.vector.tensor_scalar(
                out=sinm, in0=sine, scalar1=SIN_M, scalar2=None, op0=Alu.mult
            )
            phi = pp.tile([128, NB], F32, tag="phi")
            nc.vector.scalar_tensor_tensor(
                out=phi,
                in0=cos_t,
                scalar=COS_M,
                in1=sinm,
                op0=Alu.mult,
                op1=Alu.subtract,
            )
            onf = pp.tile([128, NB], F32, tag="onf")
            nc.vector.tensor_scalar(
                out=onf, in0=cos_t, scalar1=MARGIN_MULT, scalar2=None, op0=Alu.subtract
            )
            mask = pp.tile([128, NB], mybir.dt.uint8, tag="mask")
            nc.vector.tensor_scalar(
                out=mask, in0=cos_t, scalar1=THRESHOLD, scalar2=None, op0=Alu.is_gt
            )
            phif = pp.tile([128, NB], F32, tag="phif")
            nc.vector.select(phif, mask, phi, onf)

            nc.vector.tensor_scalar(
                out=tterm, in0=phif, scalar1=SCALE, scalar2=None, op0=Alu.mult
            )
            ec = pp.tile([128, NB], F32, tag="ec")
            nc.scalar.activation(ec, cos_t, Act.Exp, scale=SCALE, bias=mneg_b)
            ep = pp.tile([128, NB], F32, tag="ep")
            nc.scalar.activation(ep, phif, Act.Exp, scale=SCALE, bias=mneg_b)
            nc.vector.tensor_tensor(out=corr, in0=ep, in1=ec, op=Alu.subtract)

        # ---------------- Phase B: stream the weight shard -----------------
        if stage < 2:
            for b in range(NB):
                nc.vector.memset(Scols[b], 1.0)
        if stage >= 2:
            with (
                tc.tile_pool(name="wtb", bufs=1) as wp,
                tc.tile_pool(name="mmps", bufs=1, space="PSUM") as sp,
                tc.tile_pool(name="mwork", bufs=1) as mp,
            ):
                # Load the whole shard resident as bf16 (13.1MB) in a few big
                # cast-DMAs with long contiguous runs: cheap SWDGE descriptor
                # generation and deep DMA/compute pipelining.
                SUP = CS // NSUP  # classes per super-block
                wtbig = [
                    [
                        wp.tile(
                            [128, SUP],
                            BF16,
                            tag=f"wtbig{s}_{j}",
                            bufs=1,
                            name=f"wtbig{s}_{j}",
                        )
                        for j in range(ND)
                    ]
                    for s in range(NSUP)
                ]
                if "dma" not in ablate:
                    for s in range(NSUP):
                        for j in range(ND):
                            nc.gpsimd.dma_start(
                                out=wtbig[s][j][:],
                                in_=wt_h[j * 128 : (j + 1) * 128, s * SUP : (s + 1) * SUP],
                            )
                else:
                    for s in range(NSUP):
                        for j in range(ND):
                            nc.vector.memset(wtbig[s][j][:], 0.5)

                BPS = SUP // CB  # blocks per super-block

                # ---- software pipeline ----
                # prep_super: squares + ones-matmul colsums + batched rsqrt
                # prep_block: replicate inv row + normalize (wn tiles)
                # pairs of blocks are consumed lagging 2 blocks behind prep;
                # exps lag the matmuls by one PSUM tile (global pend queue).
                invrow_bf = []
                sq_of = {}

                def emit_sq(s):
                    # squares + pairwise pre-add on DVE: only 2 ones-matmul
                    # layers are needed on the PE instead of 4
                    adds = []
                    for j in range(2):
                        sq_a = mp.tile(
                            [128, SUP], BF16, tag="sqs", bufs=2, name=f"sqs{s}_{j}a"
                        )
                        sq_b = mp.tile(
                            [128, SUP], BF16, tag="sqs", bufs=2, name=f"sqs{s}_{j}b"
                        )
                        nc.vector.tensor_tensor(
                            out=sq_a,
                            in0=wtbig[s][2 * j],
                            in1=wtbig[s][2 * j],
                            op=Alu.mult,
                        )
                        nc.vector.tensor_tensor(
                            out=sq_b,
                            in0=wtbig[s][2 * j + 1],
                            in1=wtbig[s][2 * j + 1],
                            op=Alu.mult,
                        )
                        add = mp.tile(
                            [128, SUP], BF16, tag="sqadd", bufs=3, name=f"sqa{s}_{j}"
                        )
                        nc.vector.tensor_tensor(
                            out=add, in0=sq_a, in1=sq_b, op=Alu.add
                        )
                        adds.append(add)
                    sq_of[s] = adds

                def prep_super(s):
                    sqs = sq_of.pop(s)
                    n2row = mp.tile([1, SUP], F32, tag="n2row", bufs=1, name="n2row")
                    for q in range(BPS):
                        n2 = sp.tile(
                            [1, CB],
                            F32,
                            tag="n2",
                            bufs=2 if USE_PBCAST else 1,
                            name="n2",
                        )
                        for j in range(2):
                            nc.tensor.matmul(
                                n2,
                                lhsT=ones_bf,
                                rhs=sqs[j][:, q * CB : (q + 1) * CB],
                                start=(j == 0),
                                stop=(j == 1),
                            )
                        nc.scalar.activation(
                            n2row[:, q * CB : (q + 1) * CB], n2, Act.Copy
                        )
                    lnrow = mp.tile([1, SUP], F32, tag="lnrow", bufs=1, name="lnrow")
                    nc.scalar.activation(lnrow, n2row, Act.Ln, bias=eps_b[0:1, :])
                    inv_s = mp.tile([1, SUP], BF16, tag="invrow", bufs=2, name="inv_s")
                    nc.scalar.activation(inv_s, lnrow, Act.Exp, scale=-0.5)
                    invrow_bf.append(inv_s)

                def prep_block(blk, wn, h):
                    """Normalize block `blk` into half `h` of the pair tile
                    `wn` (free layout (j, h, c) so each j-slice is a
                    contiguous (128, 2*CB) matmul rhs)."""
                    s, off = blk // BPS, (blk % BPS) * CB
                    wslice = [wtbig[s][j][:, off : off + CB] for j in range(ND)]
                    invrep = mp.tile(
                        [128, CB], BF16, tag="invrepb", bufs=3, name="invrep"
                    )
                    if USE_PBCAST:
                        # replicate the inv-norm row on the (otherwise idle)
                        # GPSIMD engine: frees a PSUM bank, a PE matmul and a
                        # DVE evacuation copy per block
                        nc.gpsimd.partition_broadcast(
                            invrep[:], invrow_bf[s][0:1, off : off + CB]
                        )
                    else:
                        invrep_ps = sp.tile(
                            [128, CB], F32, tag="invrep", bufs=1, name="invrep_ps"
                        )
                        nc.tensor.matmul(
                            invrep_ps,
                            lhsT=ones_k1,
                            rhs=invrow_bf[s][:, off : off + CB],
                            start=True,
                            stop=True,
                        )
                        nc.scalar.activation(invrep, invrep_ps, Act.Copy)
                    for j in range(ND):
                        nc.vector.tensor_tensor(
                            out=wn[:, (2 * j + h) * CB : (2 * j + h + 1) * CB],
                            in0=wslice[j],
                            in1=invrep,
                            op=Alu.mult,
                        )

                pend = []

                def consume_pair(wn, width, pidx):
                    for b in range(NB):
                        g = sp.tile([128, 2 * CB], F32, tag="g", bufs=3, name="g")
                        for h in range(width // CB):
                            for j in range(ND):
                                nc.tensor.matmul(
                                    g[:, h * CB : (h + 1) * CB],
                                    lhsT=eT_bf[j][:, b * 128 : (b + 1) * 128],
                                    rhs=wn[:, (2 * j + h) * CB : (2 * j + h + 1) * CB],
                                    start=(j == 0),
                                    stop=(j == ND - 1),
                                )
                        pend.append((g, b, pidx, width))
                        while len(pend) > 2:
                            _emit_exp(nc, mp, pend.pop(0), Scols, mneg_b)

                def new_wn():
                    wn = wp.tile(
                        [128, ND * 2 * CB], BF16, tag="wn", bufs=3, name="wn"
                    )
                    return wn

                wn_q = []
                cur = None
                pidx = 0
                emit_sq(0)
                for s in range(NSUP):
                    prep_super(s)
                    for q in range(BPS):
                        if q == 0 and s + 1 < NSUP:
                            emit_sq(s + 1)  # early: DVE fills gaps this super
                        blk = s * BPS + q
                        if blk % 2 == 0:
                            cur = new_wn()
                        prep_block(blk, cur, blk % 2)
                        if blk % 2 == 1:
                            wn_q.append(cur)
                        while len(wn_q) >= 2:
                            consume_pair(wn_q.pop(0), 2 * CB, pidx)
                            pidx += 1
                while wn_q:
                    consume_pair(wn_q.pop(0), 2 * CB, pidx)
                    pidx += 1
                if NBLK % 2 == 1:
                    consume_pair(cur, CB, pidx)
                while pend:
                    _emit_exp(nc, mp, pend.pop(0), Scols, mneg_b)

        # ---------------- Phase C: all-reduce + final loss ------------------
        with (
            tc.tile_pool(name="fin", bufs=1) as fp,
            tc.tile_pool(name="fin_ps", bufs=1, space="PSUM") as fpp,
            tc.tile_pool(name="cc", bufs=1, space="DRAM") as dp,
        ):
            spart = fp.tile([128, NB], F32, tag="spart")
            for b in range(NB):
                nc.vector.tensor_reduce(
                    out=spart[:, b : b + 1], in_=Scols[b], axis=AX.X, op=Alu.add
                )
            stot = fp.tile([128, NB], F32, tag="stot")
            if stage >= 3:
                cc_in = dp.tile([128, NB], F32, tag="cc_in")
                cc_out = dp.tile([128, NB], F32, tag="cc_out", addr_space="Shared")
                nc.sync.dma_start(out=cc_in[:], in_=spart[:])
                nc.gpsimd.collective_compute(
                    "AllReduce",
                    Alu.add,
                    ins=[cc_in.opt()],
                    outs=[cc_out.opt()],
                    replica_groups=[list(range(NCORES))],
                )
                nc.sync.dma_start(out=stot[:], in_=cc_out[:])
            else:
                nc.vector.tensor_copy(out=stot, in_=spart)

            scorr = fp.tile([128, NB], F32, tag="scorr")
            nc.vector.tensor_tensor(out=scorr, in0=stot, in1=corr, op=Alu.add)
            logs = fp.tile([128, NB], F32, tag="logs")
            nc.scalar.activation(logs, scorr, Act.Ln)
            lossb = fp.tile([128, NB], F32, tag="lossb")
            nc.vector.scalar_tensor_tensor(
                out=lossb,
                in0=logs,
                scalar=MSHIFT,
                in1=tterm,
                op0=Alu.add,
                op1=Alu.subtract,
            )
            lpart = fp.tile([128, 1], F32, tag="lpart")
            nc.vector.tensor_reduce(out=lpart, in_=lossb, axis=AX.X, op=Alu.add)
            tot_ps = fpp.tile([1, 1], F32, tag="tot")
            nc.tensor.matmul(tot_ps, lhsT=lpart, rhs=ones_f, start=True, stop=True)
            outsb = fp.tile([1, 1], F32, tag="outsb")
            nc.scalar.activation(outsb, tot_ps, Act.Copy, scale=1.0 / B)
            nc.sync.dma_start(out=out_h[:], in_=outsb[:])


# ---------------------------------------------------------------------------
_NC_CACHE = None


def _get_nc():
    global _NC_CACHE
    if _NC_CACHE is None:
        _NC_CACHE = build_kernel()
    return _NC_CACHE


def make_in_maps(embeddings, labels, weight):
    e = np.ascontiguousarray(embeddings, dtype=np.float32)
    w = np.asarray(weight, dtype=np.float32)
    lab = np.asarray(labels).astype(np.int64)
    wtgt = np.ascontiguousarray(w[lab], dtype=np.float32)
    # transpose + zero-pad the class dim, then shard it
    wt_full = np.zeros((D, NCORES * CS), dtype=np.float32)
    wt_full[:, :C] = w.T
    in_maps = []
    for i in range(NCORES):
        in_maps.append(
            {
                "e": e,
                "wtgt": wtgt,
                "wt": np.ascontiguousarray(wt_full[:, i * CS : (i + 1) * CS]),
            }
        )
    return in_maps


def kernel(embeddings, labels, weight, _trace=False):
    import time as _time

    nc = _get_nc()
    in_maps = make_in_maps(embeddings, labels, weight)
    last = None
    for attempt in range(4):
        try:
            res = run_bass_kernel_spmd(
                nc, in_maps, core_ids=list(range(NCORES)), trace=_trace
            )
            break
        except Exception as ex:  # transient axon/device hiccups: retry
            last = ex
            _time.sleep(5 * (attempt + 1))
    else:
        raise last
    out = np.asarray(res.results[0]["out"], dtype=np.float32).reshape(())
    if _trace:
        return out, res
    return out



# revision 21
# speedup vs baseline: 1.3874x; 1.3874x over previous
"""ArcFace loss kernel for 8 Trainium2 NeuronCores (class-parallel sharding).

Strategy (per sharding hint): shard the class dimension of the weight matrix
across the 8 cores (12.5k classes each, padded to 12800). Each core streams
its weight shard once from HBM (the memory roofline), computes the per-shard
plain-softmax partial sums S_b = sum_c exp(s*cos(b,c) - s), and one 2KB
AllGather + local add combines them. The ArcFace margin only touches one
(b, target) element per row, so the margin correction + final loss are
computed replicated on every core from the (host-gathered) target weight
rows.

Host-side transforms are layout/dtype only: transpose + pad + bf16-cast +
super-block tiling of the weight shard (the kernel consumed bf16 weights
via cast-DMA before; pre-casting halves HBM traffic). All arithmetic of
the reference loss happens on device.

Schedule: the 5 super-block weight DMAs are issued at t=0 across all three
DMA queues (SP/Act HWDGE + Pool SWDGE) into an always-open resident pool,
so the load overlaps the embedding prep. Per-class inverse norms are
computed with a ones[128,128] matmul (column sums land replicated across
all partitions in PSUM), so the rsqrt (ln/exp pair) runs at full width and
no separate replication matmul or PSUM-evacuation copy is needed.
"""

import sys

sys.path.insert(0, "/opt/trn_rl_repo")

import math

import numpy as np

import concourse.bass as bass
import concourse.bacc as bacc
import concourse.mybir as mybir
import concourse.tile as tile
from concourse.masks import make_identity
from concourse.bass_utils import run_bass_kernel_spmd

# ---- problem constants (hardcoded per the task contract) ----
B = 512  # batch
D = 512  # embedding dim
C = 100000  # num classes
NCORES = 8
CS = 12800  # padded classes per core (8*12800 = 102400 >= 100000)
CB = 512  # class block (free dim per psum bank)
NBLK = CS // CB  # 25
NB = B // 128  # 4 batch chunks
ND = D // 128  # 4 contraction chunks
NSUP = 5  # super-blocks of the resident bf16 weight load
SUP = CS // NSUP  # 2560 classes per super-block
BPS = SUP // CB  # 5 blocks per super-block
NPAIR = (NBLK + 1) // 2  # 13 exp groups (pairs of class blocks)

SCALE = 64.0
MARGIN = 0.5
COS_M = math.cos(MARGIN)
SIN_M = math.sin(MARGIN)
THRESHOLD = math.cos(math.pi - MARGIN)
MARGIN_MULT = math.sin(math.pi - MARGIN) * MARGIN
MSHIFT = 14.0  # fixed logsumexp shift; exp args land in the well-fitted spline region
# (still overflow-safe: max arg = 64-14 = 50 -> e^50 ~ 5e21, far below f32 max)

F32 = mybir.dt.float32
BF16 = mybir.dt.bfloat16
Act = mybir.ActivationFunctionType
Alu = mybir.AluOpType
AX = mybir.AxisListType


def _ttr(nc, out, in0, in1, accum):
    """accum = sum_free(in0 * in1).

    (InstTensorTensorReduce hangs the runtime in this environment, so use a
    plain mult + free-axis reduce pair instead.)
    """
    nc.vector.tensor_tensor(out=out, in0=in0, in1=in1, op=Alu.mult)
    nc.vector.tensor_reduce(out=accum, in_=out, axis=AX.X, op=Alu.add)


def _patch_act_tables():
    """Restrict the ACT-table chooser to the combined ln+exp set.

    The default chooser pairs Ln with `natural_log` and Exp with
    `exp_and_others`, reloading tables (~2.7us each) every block.  All
    activations used here (Ln, Exp, Copy) live in
    `natural_log_exp_and_others`, so force that single set.
    """
    if getattr(bacc, "_act_tables_patched", False):
        return
    orig = bacc.get_activation_tables

    def prefer_combined(arch):
        t = orig(arch)
        if "natural_log_exp_and_others" not in t:
            return t
        out = {}
        strip = {Act.Exp, Act.Ln}
        for k, v in t.items():
            if k == "natural_log_exp_and_others":
                out[k] = v
            else:
                out[k] = {f for f in v if f not in strip}
        return out

    bacc.get_activation_tables = prefer_combined
    bacc._act_tables_patched = True


def build_kernel(stage=3, repeat=1, ablate=(), dbg=False):
    _patch_act_tables()
    nc = bacc.Bacc("TRN2", target_bir_lowering=False, debug=False, num_devices=NCORES)

    e_h = nc.declare_dram_parameter("e", [128, NB * D], BF16, isOutput=False)
    wtgt_h = nc.declare_dram_parameter("wtgt", [128, NB * D], BF16, isOutput=False)
    wt_h = nc.declare_dram_parameter("wt", [NSUP * 128, ND * SUP], BF16, isOutput=False)
    out_h = nc.declare_dram_parameter("out", [1, 1], F32, isOutput=True)

    with tile.TileContext(nc) as tc:
        for _ in range(repeat):
            _body(nc, tc, e_h, wtgt_h, wt_h, out_h, stage, set(ablate), dbg)
    nc.compile()
    return nc


def _body(nc, tc, e_h, wtgt_h, wt_h, out_h, stage=3, ablate=frozenset(), dbg=False):
    def dbg_out(name, ap):
        if not dbg:
            return
        h = nc.declare_dram_parameter(
            f"dbg_{name}", list(ap.shape), ap.dtype, isOutput=True
        )
        nc.sync.dma_start(out=h[:], in_=ap)
    with (
        tc.tile_pool(name="const", bufs=1) as cp,
        tc.tile_pool(name="wtb", bufs=1) as wp,
        tc.tile_pool(name="mwork", bufs=1) as mp,
        tc.tile_pool(name="prep", bufs=1) as pp,
        tc.tile_pool(name="fin", bufs=1) as fp,
    ):
        # ------------- DMA issue: weights + embeddings first ---------------
        # Supers load in j-pair halves so squares can start after half a
        # super lands; first half of super 0 goes first, embeddings second.
        wtbig = [
            wp.tile([128, ND * SUP], BF16, tag=f"wtbig{s}", bufs=1, name=f"wtbig{s}")
            for s in range(NSUP)
        ]
        e_sb = pp.tile([128, NB * D], BF16, tag="eload")
        wt_sb = pp.tile([128, NB * D], BF16, tag="wtload")

        def load_half(eng, s, jp):
            if "dma" in ablate:
                if jp == 0:
                    nc.vector.memset(wtbig[s][:], 0.01)
                return
            half = 2 * SUP
            eng.dma_start(
                out=wtbig[s][:, jp * half : (jp + 1) * half],
                in_=wt_h[s * 128 : (s + 1) * 128, jp * half : (jp + 1) * half],
            )

        load_half(nc.sync, 0, 0)
        nc.scalar.dma_start(out=e_sb[:], in_=e_h[:])
        load_half(nc.sync, 0, 1)
        nc.scalar.dma_start(out=wt_sb[:], in_=wtgt_h[:])
        for s in range(1, NSUP):
            for jp in range(2):
                eng = [nc.gpsimd, nc.sync, nc.scalar][(2 * s + jp) % 3]
                load_half(eng, s, jp)

        # ------------------------- constants -------------------------------
        ones128 = cp.tile([128, 128], BF16, tag="ones128")
        nc.vector.memset(ones128, 1.0)
        ones_f = cp.tile([128, 1], F32, tag="ones_f")
        nc.vector.memset(ones_f, 1.0)
        eps_b = cp.tile([128, 1], F32, tag="eps_b")
        nc.vector.memset(eps_b, 1e-24)
        mneg_b = cp.tile([128, 1], F32, tag="mneg_b")
        nc.vector.memset(mneg_b, -MSHIFT)

        # persistent state
        eT_bf = [cp.tile([128, B], BF16, tag=f"eT{j}", name=f"eT{j}") for j in range(ND)]
        e_n = [cp.tile([128, D], BF16, tag=f"en{k}", name=f"en{k}") for k in range(NB)]
        Scols = [
            cp.tile([128, NPAIR], F32, tag=f"scols{b}", name=f"scols{b}")
            for b in range(NB)
        ]
        corr = cp.tile([128, NB], F32, tag="corr")
        tterm = cp.tile([128, NB], F32, tag="tterm")

        # squares of the weight shard: sqs[jp] = w_{2jp}^2 + w_{2jp+1}^2
        def emit_squares(s):
            outs = []
            for jp in range(2):
                sq_a = mp.tile(
                    [128, SUP], BF16, tag="sqa", bufs=2, name=f"sqa{s}_{jp}"
                )
                sq_b = mp.tile(
                    [128, SUP], BF16, tag="sqb", bufs=2, name=f"sqb{s}_{jp}"
                )
                w0 = wtbig[s][:, (2 * jp) * SUP : (2 * jp + 1) * SUP]
                w1 = wtbig[s][:, (2 * jp + 1) * SUP : (2 * jp + 2) * SUP]
                nc.vector.tensor_tensor(out=sq_a, in0=w0, in1=w0, op=Alu.mult)
                nc.vector.tensor_tensor(out=sq_b, in0=w1, in1=w1, op=Alu.mult)
                add = mp.tile(
                    [128, SUP], BF16, tag="sqadd", bufs=3, name=f"sqs{s}_{jp}"
                )
                nc.vector.tensor_tensor(out=add, in0=sq_a, in1=sq_b, op=Alu.add)
                outs.append(add)
            return outs

        sqs_first = emit_squares(0) if stage >= 2 else None

        # ---------------- Phase A: embeddings + target rows (replicated) ---
        if True:
            junk = pp.tile([128, D], BF16, tag="junk", bufs=2)
            ne2 = pp.tile([128, NB], F32, tag="ne2")
            nt2 = pp.tile([128, NB], F32, tag="nt2")
            dotr = pp.tile([128, NB], F32, tag="dotr")
            for k in range(NB):
                ek = e_sb[:, k * D : (k + 1) * D]
                _ttr(nc, junk, ek, ek, ne2[:, k : k + 1])
            # inv_ne = exp(-0.5*ln(ne2 + eps^2))  (rsqrt via the ln/exp set)
            lnb = pp.tile([128, NB], F32, tag="lnb")
            nc.scalar.activation(lnb, ne2, Act.Ln, bias=eps_b)
            inv_ne = pp.tile([128, NB], F32, tag="inv_ne")
            nc.scalar.activation(inv_ne, lnb, Act.Exp, scale=-0.5)
            for k in range(NB):
                nc.vector.tensor_scalar(
                    out=e_n[k],
                    in0=e_sb[:, k * D : (k + 1) * D],
                    scalar1=inv_ne[:, k : k + 1],
                    scalar2=None,
                    op0=Alu.mult,
                )
            # eT (transposed, bf16) for the matmul stationary side, via the
            # DMA transpose XBAR (idle DMA engines; no PE/DVE cost)
            for k in range(NB):
                for j in range(ND):
                    [nc.sync, nc.scalar][(k * ND + j) % 2].dma_start(
                        out=eT_bf[j][:, k * 128 : (k + 1) * 128],
                        in_=e_n[k][:, j * 128 : (j + 1) * 128],
                        transpose=True,
                    )

            # target rows: norms + dot with e_n
            for k in range(NB):
                wk = wt_sb[:, k * D : (k + 1) * D]
                _ttr(nc, junk, wk, wk, nt2[:, k : k + 1])
            lnt = pp.tile([128, NB], F32, tag="lnt")
            nc.scalar.activation(lnt, nt2, Act.Ln, bias=eps_b)
            inv_nt = pp.tile([128, NB], F32, tag="inv_nt")
            nc.scalar.activation(inv_nt, lnt, Act.Exp, scale=-0.5)
            for k in range(NB):
                _ttr(
                    nc,
                    junk,
                    e_n[k],
                    wt_sb[:, k * D : (k + 1) * D],
                    dotr[:, k : k + 1],
                )
            cos_t = pp.tile([128, NB], F32, tag="cos_t")
            nc.vector.tensor_tensor(out=cos_t, in0=dotr, in1=inv_nt, op=Alu.mult)
            dbg_out("e_sb", e_sb[:])
            dbg_out("wt_sb", wt_sb[:])
            dbg_out("ne2", ne2[:])
            dbg_out("inv_ne", inv_ne[:])
            dbg_out("en0", e_n[0][:])
            dbg_out("eT0", eT_bf[0][:])
            dbg_out("dotr", dotr[:])
            dbg_out("cos_t", cos_t[:])

            # phi = cos*cos_m - sin*sin_m ; easy-margin=False branch
            c2 = pp.tile([128, NB], F32, tag="c2")
            nc.vector.tensor_tensor(out=c2, in0=cos_t, in1=cos_t, op=Alu.mult)
            om = pp.tile([128, NB], F32, tag="om")
            nc.vector.tensor_scalar(
                out=om, in0=c2, scalar1=-1.0, scalar2=1.0, op0=Alu.mult, op1=Alu.add
            )
            nc.vector.tensor_scalar(
                out=om, in0=om, scalar1=1e-30, scalar2=None, op0=Alu.max
            )
            lnz = pp.tile([128, NB], F32, tag="lnz")
            nc.scalar.activation(lnz, om, Act.Ln)
            sine = pp.tile([128, NB], F32, tag="sine")
            nc.scalar.activation(sine, lnz, Act.Exp, scale=0.5)
            sinm = pp.tile([128, NB], F32, tag="sinm")
            nc.vector.tensor_scalar(
                out=sinm, in0=sine, scalar1=SIN_M, scalar2=None, op0=Alu.mult
            )
            phi = pp.tile([128, NB], F32, tag="phi")
            nc.vector.scalar_tensor_tensor(
                out=phi,
                in0=cos_t,
                scalar=COS_M,
                in1=sinm,
                op0=Alu.mult,
                op1=Alu.subtract,
            )
            onf = pp.tile([128, NB], F32, tag="onf")
            nc.vector.tensor_scalar(
                out=onf, in0=cos_t, scalar1=MARGIN_MULT, scalar2=None, op0=Alu.subtract
            )
            mask = pp.tile([128, NB], mybir.dt.uint8, tag="mask")
            nc.vector.tensor_scalar(
                out=mask, in0=cos_t, scalar1=THRESHOLD, scalar2=None, op0=Alu.is_gt
            )
            phif = pp.tile([128, NB], F32, tag="phif")
            nc.vector.select(phif, mask, phi, onf)

            nc.vector.tensor_scalar(
                out=tterm, in0=phif, scalar1=SCALE, scalar2=None, op0=Alu.mult
            )
            ec = pp.tile([128, NB], F32, tag="ec")
            nc.scalar.activation(ec, cos_t, Act.Exp, scale=SCALE, bias=mneg_b)
            ep = pp.tile([128, NB], F32, tag="ep")
            nc.scalar.activation(ep, phif, Act.Exp, scale=SCALE, bias=mneg_b)
            nc.vector.tensor_tensor(out=corr, in0=ep, in1=ec, op=Alu.subtract)

        # ---------------- Phase B: normalize + matmul + exp stream ---------
        if stage < 2:
            for b in range(NB):
                nc.vector.memset(Scols[b], 1.0)
        if stage >= 2:
            with tc.tile_pool(name="mmps", bufs=1, space="PSUM") as sp:
                pend = []

                def emit_exp(item):
                    g, b, pidx, width = item
                    esc = mp.tile([128, 2 * CB], BF16, tag="esc", bufs=2, name="esc")
                    nc.scalar.activation(
                        esc[:, 0:width],
                        g[:, 0:width],
                        Act.Exp,
                        scale=SCALE,
                        bias=mneg_b,
                        accum_out=Scols[b][:, pidx : pidx + 1],
                    )

                def prep_block(s, q, sqs, wn, h):
                    """Normalize block q of super s into half h of pair tile
                    `wn` (free layout (j, h, c) so each j-slice is a
                    contiguous (128, 2*CB) matmul rhs)."""
                    off = q * CB
                    # column sums of squares, replicated over all partitions
                    n2 = sp.tile([128, CB], F32, tag="n2", bufs=2, name="n2")
                    for jp in range(2):
                        nc.tensor.matmul(
                            n2,
                            lhsT=ones128,
                            rhs=sqs[jp][:, off : off + CB],
                            start=(jp == 0),
                            stop=(jp == 1),
                        )
                    # inv_norm = exp(-0.5 * ln(n2 + eps^2)), full width
                    lnv = mp.tile([128, CB], F32, tag="lnv", bufs=2, name="lnv")
                    nc.scalar.activation(lnv, n2, Act.Ln, bias=eps_b)
                    invrep = mp.tile(
                        [128, CB], BF16, tag="invrep", bufs=3, name="invrep"
                    )
                    nc.scalar.activation(invrep, lnv, Act.Exp, scale=-0.5)
                    for j in range(ND):
                        nc.vector.tensor_tensor(
                            out=wn[:, (2 * j + h) * CB : (2 * j + h + 1) * CB],
                            in0=wtbig[s][:, j * SUP + off : j * SUP + off + CB],
                            in1=invrep,
                            op=Alu.mult,
                        )

                def consume_pair(wn, width, pidx):
                    for b in range(NB):
                        g = sp.tile([128, 2 * CB], F32, tag="g", bufs=3, name="g")
                        for h in range(width // CB):
                            for j in range(ND):
                                nc.tensor.matmul(
                                    g[:, h * CB : (h + 1) * CB],
                                    lhsT=eT_bf[j][:, b * 128 : (b + 1) * 128],
                                    rhs=wn[:, (2 * j + h) * CB : (2 * j + h + 1) * CB],
                                    start=(j == 0),
                                    stop=(j == ND - 1),
                                )
                        pend.append((g, b, pidx, width))
                        while len(pend) > 2:
                            emit_exp(pend.pop(0))

                wn_q = []
                cur = None
                pidx = 0
                for s in range(NSUP):
                    sqs = sqs_first if s == 0 else emit_squares(s)
                    for q in range(BPS):
                        blk = s * BPS + q
                        if blk % 2 == 0:
                            cur = wp.tile(
                                [128, ND * 2 * CB], BF16, tag="wn", bufs=3, name="wn"
                            )
                        prep_block(s, q, sqs, cur, blk % 2)
                        if blk % 2 == 1:
                            wn_q.append(cur)
                        while len(wn_q) >= 2:
                            consume_pair(wn_q.pop(0), 2 * CB, pidx)
                            pidx += 1
                while wn_q:
                    consume_pair(wn_q.pop(0), 2 * CB, pidx)
                    pidx += 1
                if NBLK % 2 == 1:
                    consume_pair(cur, CB, pidx)
                while pend:
                    emit_exp(pend.pop(0))

        # ---------------- Phase C: all-gather + final loss ------------------
        with (
            tc.tile_pool(name="fin_ps", bufs=1, space="PSUM") as fpp,
            tc.tile_pool(name="cc", bufs=1, space="DRAM") as dp,
        ):
            spart = fp.tile([128, NB], F32, tag="spart")
            for b in range(NB):
                nc.vector.tensor_reduce(
                    out=spart[:, b : b + 1], in_=Scols[b], axis=AX.X, op=Alu.add
                )
            stot = fp.tile([128, NB], F32, tag="stot")
            if stage >= 3:
                cc_in = dp.tile([1, 128 * NB], F32, tag="cc_in")
                cc_out = dp.tile(
                    [1, NCORES * 128 * NB], F32, tag="cc_out", addr_space="Shared"
                )
                nc.sync.dma_start(
                    out=cc_in[:].rearrange("1 (p n) -> p n", p=128), in_=spart[:]
                )
                nc.gpsimd.collective_compute(
                    "AllGather",
                    Alu.bypass,
                    ins=[cc_in.opt()],
                    outs=[cc_out.opt()],
                    replica_groups=[list(range(NCORES))],
                )
                gath = fp.tile([128, NCORES * NB], F32, tag="gath")
                nc.sync.dma_start(
                    out=gath[:].rearrange("p (r n) -> p r n", r=NCORES),
                    in_=cc_out[:].rearrange("1 (r p n) -> p r n", r=NCORES, p=128),
                )
                # tree-reduce the 8 rank slices
                t1 = fp.tile([128, 4 * NB], F32, tag="t1")
                nc.vector.tensor_tensor(
                    out=t1, in0=gath[:, : 4 * NB], in1=gath[:, 4 * NB :], op=Alu.add
                )
                t2 = fp.tile([128, 2 * NB], F32, tag="t2")
                nc.vector.tensor_tensor(
                    out=t2, in0=t1[:, : 2 * NB], in1=t1[:, 2 * NB :], op=Alu.add
                )
                nc.vector.tensor_tensor(
                    out=stot, in0=t2[:, :NB], in1=t2[:, NB:], op=Alu.add
                )
            else:
                nc.vector.tensor_copy(out=stot, in_=spart)

            scorr = fp.tile([128, NB], F32, tag="scorr")
            nc.vector.tensor_tensor(out=scorr, in0=stot, in1=corr, op=Alu.add)
            logs = fp.tile([128, NB], F32, tag="logs")
            nc.scalar.activation(logs, scorr, Act.Ln)
            lossb = fp.tile([128, NB], F32, tag="lossb")
            nc.vector.scalar_tensor_tensor(
                out=lossb,
                in0=logs,
                scalar=MSHIFT,
                in1=tterm,
                op0=Alu.add,
                op1=Alu.subtract,
            )
            lpart = fp.tile([128, 1], F32, tag="lpart")
            nc.vector.tensor_reduce(out=lpart, in_=lossb, axis=AX.X, op=Alu.add)
            tot_ps = fpp.tile([1, 1], F32, tag="tot")
            nc.tensor.matmul(tot_ps, lhsT=lpart, rhs=ones_f, start=True, stop=True)
            outsb = fp.tile([1, 1], F32, tag="outsb")
            nc.scalar.activation(outsb, tot_ps, Act.Copy, scale=1.0 / B)
            nc.sync.dma_start(out=out_h[:], in_=outsb[:])


# ---------------------------------------------------------------------------
_NC_CACHE = None


def _get_nc():
    global _NC_CACHE
    if _NC_CACHE is None:
        _NC_CACHE = build_kernel()
    return _NC_CACHE


def make_in_maps(embeddings, labels, weight):
    import ml_dtypes

    def fold(x):  # [B, D] -> [128, NB*D] with batch b = k*128 + p
        return np.ascontiguousarray(
            x.reshape(NB, 128, D).transpose(1, 0, 2).reshape(128, NB * D)
        )

    e = fold(np.asarray(embeddings, dtype=np.float32).astype(ml_dtypes.bfloat16))
    w = np.asarray(weight, dtype=np.float32)
    lab = np.asarray(labels).astype(np.int64)
    wtgt = fold(w[lab].astype(ml_dtypes.bfloat16))
    # transpose + zero-pad the class dim + bf16 cast, then shard + tile it
    wt_full = np.zeros((D, NCORES * CS), dtype=ml_dtypes.bfloat16)
    wt_full[:, :C] = w.T
    in_maps = []
    for i in range(NCORES):
        shard = wt_full[:, i * CS : (i + 1) * CS]  # [D, CS]
        tiled = (
            shard.reshape(ND, 128, NSUP, SUP)
            .transpose(2, 1, 0, 3)
            .reshape(NSUP * 128, ND * SUP)
        )
        in_maps.append(
            {
                "e": e,
                "wtgt": wtgt,
                "wt": np.ascontiguousarray(tiled),
            }
        )
    return in_maps


def kernel(embeddings, labels, weight, _trace=False):
    import time as _time

    nc = _get_nc()
    in_maps = make_in_maps(embeddings, labels, weight)
    last = None
    for attempt in range(4):
        try:
            res = run_bass_kernel_spmd(
                nc, in_maps, core_ids=list(range(NCORES)), trace=_trace
            )
            break
        except Exception as ex:  # transient axon/device hiccups: retry
            last = ex
            _time.sleep(5 * (attempt + 1))
    else:
        raise last
    out = np.asarray(res.results[0]["out"], dtype=np.float32).reshape(())
    if _trace:
        return out, res
    return out


# revision 34
# speedup vs baseline: 1.7232x; 1.2421x over previous
"""ArcFace loss kernel for 8 Trainium2 NeuronCores (class-parallel sharding).

Strategy (per sharding hint): shard the class dimension of the weight matrix
across the 8 cores (12.5k classes each, padded to 12800). Each core streams
its weight shard once from HBM (the memory roofline), computes the per-shard
plain-softmax partial sums S_b = sum_c exp(s*cos(b,c) - s), and one 2KB
AllGather + local add combines them. The ArcFace margin only touches one
(b, target) element per row, so the margin correction + final loss are
computed replicated on every core from the (host-gathered) target weight
rows.

Host-side transforms are layout/dtype only: transpose + pad + bf16-cast +
super-block tiling of the weight shard (the kernel consumed bf16 weights
via cast-DMA before; pre-casting halves HBM traffic). All arithmetic of
the reference loss happens on device.

Schedule: the 5 super-block weight DMAs are issued at t=0 across all three
DMA queues (SP/Act HWDGE + Pool SWDGE) into an always-open resident pool,
so the load overlaps the embedding prep. Per-class inverse norms are
computed with a ones[128,128] matmul (column sums land replicated across
all partitions in PSUM), so the rsqrt (ln/exp pair) runs at full width and
no separate replication matmul or PSUM-evacuation copy is needed.
"""

import sys

sys.path.insert(0, "/opt/trn_rl_repo")

import math

import numpy as np

import concourse.bass as bass
import concourse.bacc as bacc
import concourse.mybir as mybir
import concourse.tile as tile
from concourse.masks import make_identity
from concourse.bass_utils import run_bass_kernel_spmd

# ---- problem constants (hardcoded per the task contract) ----
B = 512  # batch
D = 512  # embedding dim
C = 100000  # num classes
NCORES = 8
CS = 12800  # padded classes per core (8*12800 = 102400 >= 100000)
CB = 512  # class block (free dim per psum bank)
NBLK = CS // CB  # 25
NB = B // 128  # 4 batch chunks
ND = D // 128  # 4 contraction chunks
NSUP = 5  # super-blocks of the resident bf16 weight load
SUP = CS // NSUP  # 2560 classes per super-block
BPS = SUP // CB  # 5 blocks per super-block
NPAIR = (NBLK + 1) // 2  # 13 exp groups (pairs of class blocks)

SCALE = 64.0
MARGIN = 0.5
COS_M = math.cos(MARGIN)
SIN_M = math.sin(MARGIN)
THRESHOLD = math.cos(math.pi - MARGIN)
MARGIN_MULT = math.sin(math.pi - MARGIN) * MARGIN
MSHIFT = 14.0  # fixed logsumexp shift; exp args land in the well-fitted spline region
# (still overflow-safe: max arg = 64-14 = 50 -> e^50 ~ 5e21, far below f32 max)

F32 = mybir.dt.float32
BF16 = mybir.dt.bfloat16
FP8 = mybir.dt.float8e4
Act = mybir.ActivationFunctionType
Alu = mybir.AluOpType
AX = mybir.AxisListType
DR = mybir.MatmulPerfMode.DoubleRow
FP8_SCALE = 16.0  # both matmul operands are scaled x16 (folded into inv-norms)
LN_S = math.log(FP8_SCALE)


def _ttr(nc, out, in0, in1, accum):
    """accum = sum_free(in0 * in1).

    (InstTensorTensorReduce hangs the runtime in this environment, so use a
    plain mult + free-axis reduce pair instead.)
    """
    nc.vector.tensor_tensor(out=out, in0=in0, in1=in1, op=Alu.mult)
    nc.vector.tensor_reduce(out=accum, in_=out, axis=AX.X, op=Alu.add)


def _patch_act_tables():
    """Restrict the ACT-table chooser to the combined ln+exp set.

    The default chooser pairs Ln with `natural_log` and Exp with
    `exp_and_others`, reloading tables (~2.7us each) every block.  All
    activations used here (Ln, Exp, Copy) live in
    `natural_log_exp_and_others`, so force that single set.
    """
    if getattr(bacc, "_act_tables_patched", False):
        return
    orig = bacc.get_activation_tables

    def prefer_combined(arch):
        t = orig(arch)
        if "natural_log_exp_and_others" not in t:
            return t
        out = {}
        strip = {Act.Exp, Act.Ln}
        for k, v in t.items():
            if k == "natural_log_exp_and_others":
                out[k] = v
            else:
                out[k] = {f for f in v if f not in strip}
        return out

    bacc.get_activation_tables = prefer_combined
    bacc._act_tables_patched = True


def build_kernel(stage=3, repeat=1, ablate=(), dbg=False, fp8=True):
    _patch_act_tables()
    nc = bacc.Bacc("TRN2", target_bir_lowering=False, debug=False, num_devices=NCORES)

    e_h = nc.declare_dram_parameter("e", [128, NB * D], BF16, isOutput=False)
    wtgt_h = nc.declare_dram_parameter("wtgt", [128, NB * D], BF16, isOutput=False)
    wt_h = nc.declare_dram_parameter("wt", [NSUP * 128, ND * SUP], BF16, isOutput=False)
    out_h = nc.declare_dram_parameter("out", [1, 1], F32, isOutput=True)

    with tile.TileContext(nc) as tc:
        for _ in range(repeat):
            _body(nc, tc, e_h, wtgt_h, wt_h, out_h, stage, set(ablate), dbg, fp8)
    nc.compile()
    return nc


def _body(
    nc, tc, e_h, wtgt_h, wt_h, out_h, stage=3, ablate=frozenset(), dbg=False, fp8=True
):
    def dbg_out(name, ap):
        if not dbg:
            return
        h = nc.declare_dram_parameter(
            f"dbg_{name}", list(ap.shape), ap.dtype, isOutput=True
        )
        nc.sync.dma_start(out=h[:], in_=ap)
    with (
        tc.tile_pool(name="const", bufs=1) as cp,
        tc.tile_pool(name="wtb", bufs=1) as wp,
        tc.tile_pool(name="mwork", bufs=1) as mp,
        tc.tile_pool(name="prep", bufs=1) as pp,
        tc.tile_pool(name="fin", bufs=1) as fp,
    ):
        # ------------- DMA issue: weights + embeddings first ---------------
        # Supers load in j-pair halves so squares can start after half a
        # super lands; first half of super 0 goes first, embeddings second.
        wtbig = [
            wp.tile([128, ND * SUP], BF16, tag=f"wtbig{s}", bufs=1, name=f"wtbig{s}")
            for s in range(NSUP)
        ]
        e_sb = pp.tile([128, NB * D], BF16, tag="eload")
        wt_sb = pp.tile([128, NB * D], BF16, tag="wtload")

        def load_half(eng, s, jp):
            if "dma" in ablate:
                if jp == 0:
                    nc.vector.memset(wtbig[s][:], 0.01)
                return
            half = 2 * SUP
            eng.dma_start(
                out=wtbig[s][:, jp * half : (jp + 1) * half],
                in_=wt_h[s * 128 : (s + 1) * 128, jp * half : (jp + 1) * half],
            )

        load_half(nc.sync, 0, 0)
        nc.scalar.dma_start(out=e_sb[:], in_=e_h[:])
        load_half(nc.sync, 0, 1)
        nc.scalar.dma_start(out=wt_sb[:], in_=wtgt_h[:])
        for s in range(1, NSUP):
            for jp in range(2):
                eng = [nc.gpsimd, nc.sync, nc.scalar][(2 * s + jp) % 3]
                load_half(eng, s, jp)

        # ------------------------- constants -------------------------------
        ones128 = cp.tile([128, 128], BF16, tag="ones128")
        nc.vector.memset(ones128, 1.0)
        ones_f = cp.tile([128, 1], F32, tag="ones_f")
        nc.vector.memset(ones_f, 1.0)
        eps_b = cp.tile([128, 1], F32, tag="eps_b")
        nc.vector.memset(eps_b, 1e-24)
        mneg_b = cp.tile([128, 1], F32, tag="mneg_b")
        nc.vector.memset(mneg_b, -MSHIFT)
        if fp8:
            ln16_b = cp.tile([128, 1], F32, tag="ln16_b")
            nc.vector.memset(ln16_b, LN_S)
            mln16_b = cp.tile([128, 1], F32, tag="mln16_b")
            nc.vector.memset(mln16_b, -LN_S)

        # persistent state
        eT = cp.tile([128, ND * B], BF16, tag="eT", name="eT")
        eT_f8 = cp.tile([128, ND * B], FP8, tag="eT_f8", name="eT_f8") if fp8 else None
        e_n = [cp.tile([128, D], BF16, tag=f"en{k}", name=f"en{k}") for k in range(NB)]
        Scols = [
            cp.tile([128, NPAIR], F32, tag=f"scols{b}", name=f"scols{b}")
            for b in range(NB)
        ]
        corr = cp.tile([128, NB], F32, tag="corr")
        tterm = cp.tile([128, NB], F32, tag="tterm")

        # squares of the weight shard: sqs[jp] = w_{2jp}^2 + w_{2jp+1}^2
        def emit_squares(s):
            outs = []
            for jp in range(2):
                sq_a = mp.tile(
                    [128, SUP], BF16, tag="sqa", bufs=2, name=f"sqa{s}_{jp}"
                )
                sq_b = mp.tile(
                    [128, SUP], BF16, tag="sqb", bufs=2, name=f"sqb{s}_{jp}"
                )
                w0 = wtbig[s][:, (2 * jp) * SUP : (2 * jp + 1) * SUP]
                w1 = wtbig[s][:, (2 * jp + 1) * SUP : (2 * jp + 2) * SUP]
                nc.vector.tensor_tensor(out=sq_a, in0=w0, in1=w0, op=Alu.mult)
                nc.vector.tensor_tensor(out=sq_b, in0=w1, in1=w1, op=Alu.mult)
                add = mp.tile(
                    [128, SUP], BF16, tag="sqadd", bufs=3, name=f"sqs{s}_{jp}"
                )
                nc.vector.tensor_tensor(out=add, in0=sq_a, in1=sq_b, op=Alu.add)
                outs.append(add)
            return outs

        sqs_first = emit_squares(0) if stage >= 2 else None

        # ---------------- Phase A: embeddings + target rows (replicated) ---
        if True:
            junk = pp.tile([128, D], BF16, tag="junk", bufs=2)
            ne2 = pp.tile([128, NB], F32, tag="ne2")
            nt2 = pp.tile([128, NB], F32, tag="nt2")
            dotr = pp.tile([128, NB], F32, tag="dotr")
            for k in range(NB):
                ek = e_sb[:, k * D : (k + 1) * D]
                _ttr(nc, junk, ek, ek, ne2[:, k : k + 1])
            # inv_ne = exp(-0.5*ln(ne2 + eps^2))  (rsqrt via the ln/exp set)
            lnb = pp.tile([128, NB], F32, tag="lnb")
            nc.scalar.activation(lnb, ne2, Act.Ln, bias=eps_b)
            inv_ne = pp.tile([128, NB], F32, tag="inv_ne")
            # fp8: e_n = 16*e_hat (scale folded here); cos_t path divides by
            # 16 again via inv_nt's bias below
            nc.scalar.activation(
                inv_ne, lnb, Act.Exp, scale=-0.5, bias=ln16_b if fp8 else None
            )
            for k in range(NB):
                nc.vector.tensor_scalar(
                    out=e_n[k],
                    in0=e_sb[:, k * D : (k + 1) * D],
                    scalar1=inv_ne[:, k : k + 1],
                    scalar2=None,
                    op0=Alu.mult,
                )
            # eT (transposed, bf16) for the matmul stationary side, via the
            # DMA transpose XBAR (idle DMA engines; no PE/DVE cost)
            for k in range(NB):
                for j in range(ND):
                    [nc.sync, nc.scalar][(k * ND + j) % 2].dma_start(
                        out=eT[:, j * B + k * 128 : j * B + (k + 1) * 128],
                        in_=e_n[k][:, j * 128 : (j + 1) * 128],
                        transpose=True,
                    )
            if fp8:
                nc.gpsimd.dma_start(out=eT_f8[:], in_=eT[:])

            # target rows: norms + dot with e_n
            for k in range(NB):
                wk = wt_sb[:, k * D : (k + 1) * D]
                _ttr(nc, junk, wk, wk, nt2[:, k : k + 1])
            lnt = pp.tile([128, NB], F32, tag="lnt")
            nc.scalar.activation(lnt, nt2, Act.Ln, bias=eps_b)
            inv_nt = pp.tile([128, NB], F32, tag="inv_nt")
            nc.scalar.activation(
                inv_nt, lnt, Act.Exp, scale=-0.5, bias=mln16_b if fp8 else None
            )
            for k in range(NB):
                _ttr(
                    nc,
                    junk,
                    e_n[k],
                    wt_sb[:, k * D : (k + 1) * D],
                    dotr[:, k : k + 1],
                )
            cos_t = pp.tile([128, NB], F32, tag="cos_t")
            nc.vector.tensor_tensor(out=cos_t, in0=dotr, in1=inv_nt, op=Alu.mult)
            dbg_out("e_sb", e_sb[:])
            dbg_out("wt_sb", wt_sb[:])
            dbg_out("ne2", ne2[:])
            dbg_out("inv_ne", inv_ne[:])
            dbg_out("en0", e_n[0][:])
            dbg_out("eT0", eT[:, :B])
            dbg_out("dotr", dotr[:])
            dbg_out("cos_t", cos_t[:])

            # phi = cos*cos_m - sin*sin_m ; easy-margin=False branch
            c2 = pp.tile([128, NB], F32, tag="c2")
            nc.vector.tensor_tensor(out=c2, in0=cos_t, in1=cos_t, op=Alu.mult)
            om = pp.tile([128, NB], F32, tag="om")
            nc.vector.tensor_scalar(
                out=om, in0=c2, scalar1=-1.0, scalar2=1.0, op0=Alu.mult, op1=Alu.add
            )
            nc.vector.tensor_scalar(
                out=om, in0=om, scalar1=1e-30, scalar2=None, op0=Alu.max
            )
            lnz = pp.tile([128, NB], F32, tag="lnz")
            nc.scalar.activation(lnz, om, Act.Ln)
            sine = pp.tile([128, NB], F32, tag="sine")
            nc.scalar.activation(sine, lnz, Act.Exp, scale=0.5)
            sinm = pp.tile([128, NB], F32, tag="sinm")
            nc.vector.tensor_scalar(
                out=sinm, in0=sine, scalar1=SIN_M, scalar2=None, op0=Alu.mult
            )
            phi = pp.tile([128, NB], F32, tag="phi")
            nc.vector.scalar_tensor_tensor(
                out=phi,
                in0=cos_t,
                scalar=COS_M,
                in1=sinm,
                op0=Alu.mult,
                op1=Alu.subtract,
            )
            onf = pp.tile([128, NB], F32, tag="onf")
            nc.vector.tensor_scalar(
                out=onf, in0=cos_t, scalar1=MARGIN_MULT, scalar2=None, op0=Alu.subtract
            )
            mask = pp.tile([128, NB], mybir.dt.uint8, tag="mask")
            nc.vector.tensor_scalar(
                out=mask, in0=cos_t, scalar1=THRESHOLD, scalar2=None, op0=Alu.is_gt
            )
            phif = pp.tile([128, NB], F32, tag="phif")
            nc.vector.select(phif, mask, phi, onf)

            nc.vector.tensor_scalar(
                out=tterm, in0=phif, scalar1=SCALE, scalar2=None, op0=Alu.mult
            )
            ec = pp.tile([128, NB], F32, tag="ec")
            nc.scalar.activation(ec, cos_t, Act.Exp, scale=SCALE, bias=mneg_b)
            ep = pp.tile([128, NB], F32, tag="ep")
            nc.scalar.activation(ep, phif, Act.Exp, scale=SCALE, bias=mneg_b)
            nc.vector.tensor_tensor(out=corr, in0=ep, in1=ec, op=Alu.subtract)

        # ---------------- Phase B: normalize + matmul + exp stream ---------
        if stage < 2:
            for b in range(NB):
                nc.vector.memset(Scols[b], 1.0)
        if stage >= 2:
            with tc.tile_pool(name="mmps", bufs=1, space="PSUM") as sp:
                pend = []

                exp_scale = SCALE / (FP8_SCALE * FP8_SCALE) if fp8 else SCALE

                def emit_exp(item):
                    g, b, pidx, width = item
                    esc = mp.tile([128, 2 * CB], BF16, tag="esc", bufs=2, name="esc")
                    nc.scalar.activation(
                        esc[:, 0:width],
                        g[:, 0:width],
                        Act.Exp,
                        scale=exp_scale,
                        bias=mneg_b,
                        accum_out=Scols[b][:, pidx : pidx + 1],
                    )

                def prep_block(s, q, sqs, wn, h):
                    """Normalize block q of super s into half h of pair tile
                    `wn` (free layout (j, h, c) so each j-slice is a
                    contiguous (128, 2*CB) matmul rhs)."""
                    off = q * CB
                    # column sums of squares, replicated over all partitions
                    n2 = sp.tile([128, CB], F32, tag="n2", bufs=2, name="n2")
                    for jp in range(2):
                        nc.tensor.matmul(
                            n2,
                            lhsT=ones128,
                            rhs=sqs[jp][:, off : off + CB],
                            start=(jp == 0),
                            stop=(jp == 1),
                        )
                    # inv_norm = exp(-0.5 * ln(n2 + eps^2)), full width
                    # (fp8: wn = 16 * w_hat via the ln16 bias)
                    lnv = mp.tile([128, CB], F32, tag="lnv", bufs=2, name="lnv")
                    nc.scalar.activation(lnv, n2, Act.Ln, bias=eps_b)
                    invrep = mp.tile(
                        [128, CB], BF16, tag="invrep", bufs=3, name="invrep"
                    )
                    nc.scalar.activation(
                        invrep, lnv, Act.Exp, scale=-0.5,
                        bias=ln16_b if fp8 else None,
                    )
                    for j in range(ND):
                        nc.vector.tensor_tensor(
                            out=wn[:, (2 * j + h) * CB : (2 * j + h + 1) * CB],
                            in0=wtbig[s][:, j * SUP + off : j * SUP + off + CB],
                            in1=invrep,
                            op=Alu.mult,
                        )

                def consume_pair(wn, width, pidx):
                    if fp8:
                        # cast the pair tile to fp8 on the (idle) DMA engines;
                        # layout (j,h,c) == (jp,ko,h,c) needed by DoubleRow
                        wn8 = wp.tile(
                            [128, ND * 2 * CB], FP8, tag="wn8", bufs=3, name="wn8"
                        )
                        if width == 2 * CB:
                            nc.gpsimd.dma_start(out=wn8[:], in_=wn[:])
                        else:  # lone trailing block: only h=0 halves written
                            nc.gpsimd.dma_start(
                                out=wn8.rearrange("p (j hc) -> p j hc", j=ND)[
                                    :, :, 0:CB
                                ],
                                in_=wn.rearrange("p (j hc) -> p j hc", j=ND)[
                                    :, :, 0:CB
                                ],
                            )
                        wn8v = wn8.rearrange(
                            "p (jp ko h c) -> p jp ko h c", jp=2, ko=2, h=2
                        )
                        eTv = eT_f8.rearrange("p (j b) -> p j b", j=ND)
                    for b in range(NB):
                        g = sp.tile([128, 2 * CB], F32, tag="g", bufs=3, name="g")
                        for h in range(width // CB):
                            if fp8:
                                for jp in range(2):
                                    nc.tensor.matmul(
                                        g[:, h * CB : (h + 1) * CB],
                                        lhsT=eTv[
                                            :, 2 * jp : 2 * jp + 2,
                                            b * 128 : (b + 1) * 128,
                                        ],
                                        rhs=wn8v[:, jp, :, h, :],
                                        start=(jp == 0),
                                        stop=(jp == 1),
                                        perf_mode=DR,
                                    )
                            else:
                                for j in range(ND):
                                    nc.tensor.matmul(
                                        g[:, h * CB : (h + 1) * CB],
                                        lhsT=eT[:, j * B + b * 128 : j * B + (b + 1) * 128],
                                        rhs=wn[
                                            :, (2 * j + h) * CB : (2 * j + h + 1) * CB
                                        ],
                                        start=(j == 0),
                                        stop=(j == ND - 1),
                                    )
                        pend.append((g, b, pidx, width))
                        while len(pend) > 2:
                            emit_exp(pend.pop(0))

                wn_q = []
                cur = None
                pidx = 0
                for s in range(NSUP):
                    sqs = sqs_first if s == 0 else emit_squares(s)
                    for q in range(BPS):
                        blk = s * BPS + q
                        if blk % 2 == 0:
                            cur = wp.tile(
                                [128, ND * 2 * CB], BF16, tag="wn", bufs=3, name="wn"
                            )
                        prep_block(s, q, sqs, cur, blk % 2)
                        if blk % 2 == 1:
                            wn_q.append(cur)
                        while len(wn_q) >= 2:
                            consume_pair(wn_q.pop(0), 2 * CB, pidx)
                            pidx += 1
                while wn_q:
                    consume_pair(wn_q.pop(0), 2 * CB, pidx)
                    pidx += 1
                if NBLK % 2 == 1:
                    consume_pair(cur, CB, pidx)
                while pend:
                    emit_exp(pend.pop(0))

        # ---------------- Phase C: all-gather + final loss ------------------
        with (
            tc.tile_pool(name="fin_ps", bufs=1, space="PSUM") as fpp,
            tc.tile_pool(name="cc", bufs=1, space="DRAM") as dp,
        ):
            spart = fp.tile([128, NB], F32, tag="spart")
            for b in range(NB):
                nc.vector.tensor_reduce(
                    out=spart[:, b : b + 1], in_=Scols[b], axis=AX.X, op=Alu.add
                )
            stot = fp.tile([128, NB], F32, tag="stot")
            if stage >= 3:
                cc_in = dp.tile([1, 128 * NB], F32, tag="cc_in")
                cc_out = dp.tile(
                    [1, NCORES * 128 * NB], F32, tag="cc_out", addr_space="Shared"
                )
                nc.sync.dma_start(
                    out=cc_in[:].rearrange("1 (p n) -> p n", p=128), in_=spart[:]
                )
                nc.gpsimd.collective_compute(
                    "AllGather",
                    Alu.bypass,
                    ins=[cc_in.opt()],
                    outs=[cc_out.opt()],
                    replica_groups=[list(range(NCORES))],
                )
                gath = fp.tile([128, NCORES * NB], F32, tag="gath")
                nc.sync.dma_start(
                    out=gath[:].rearrange("p (r n) -> p r n", r=NCORES),
                    in_=cc_out[:].rearrange("1 (r p n) -> p r n", r=NCORES, p=128),
                )
                # tree-reduce the 8 rank slices
                t1 = fp.tile([128, 4 * NB], F32, tag="t1")
                nc.vector.tensor_tensor(
                    out=t1, in0=gath[:, : 4 * NB], in1=gath[:, 4 * NB :], op=Alu.add
                )
                t2 = fp.tile([128, 2 * NB], F32, tag="t2")
                nc.vector.tensor_tensor(
                    out=t2, in0=t1[:, : 2 * NB], in1=t1[:, 2 * NB :], op=Alu.add
                )
                nc.vector.tensor_tensor(
                    out=stot, in0=t2[:, :NB], in1=t2[:, NB:], op=Alu.add
                )
            else:
                nc.vector.tensor_copy(out=stot, in_=spart)

            scorr = fp.tile([128, NB], F32, tag="scorr")
            nc.vector.tensor_tensor(out=scorr, in0=stot, in1=corr, op=Alu.add)
            logs = fp.tile([128, NB], F32, tag="logs")
            nc.scalar.activation(logs, scorr, Act.Ln)
            lossb = fp.tile([128, NB], F32, tag="lossb")
            nc.vector.scalar_tensor_tensor(
                out=lossb,
                in0=logs,
                scalar=MSHIFT,
                in1=tterm,
                op0=Alu.add,
                op1=Alu.subtract,
            )
            lpart = fp.tile([128, 1], F32, tag="lpart")
            nc.vector.tensor_reduce(out=lpart, in_=lossb, axis=AX.X, op=Alu.add)
            tot_ps = fpp.tile([1, 1], F32, tag="tot")
            nc.tensor.matmul(tot_ps, lhsT=lpart, rhs=ones_f, start=True, stop=True)
            outsb = fp.tile([1, 1], F32, tag="outsb")
            nc.scalar.activation(outsb, tot_ps, Act.Copy, scale=1.0 / B)
            nc.sync.dma_start(out=out_h[:], in_=outsb[:])


# ---------------------------------------------------------------------------
_NC_CACHE = None


def _get_nc():
    global _NC_CACHE
    if _NC_CACHE is None:
        _NC_CACHE = build_kernel()
    return _NC_CACHE


def make_in_maps(embeddings, labels, weight):
    import ml_dtypes

    def fold(x):  # [B, D] -> [128, NB*D] with batch b = k*128 + p
        return np.ascontiguousarray(
            x.reshape(NB, 128, D).transpose(1, 0, 2).reshape(128, NB * D)
        )

    e = fold(np.asarray(embeddings, dtype=np.float32).astype(ml_dtypes.bfloat16))
    w = np.asarray(weight, dtype=np.float32)
    lab = np.asarray(labels).astype(np.int64)
    wtgt = fold(w[lab].astype(ml_dtypes.bfloat16))
    # transpose + zero-pad the class dim + bf16 cast, then shard + tile it
    wt_full = np.zeros((D, NCORES * CS), dtype=ml_dtypes.bfloat16)
    wt_full[:, :C] = w.T
    in_maps = []
    for i in range(NCORES):
        shard = wt_full[:, i * CS : (i + 1) * CS]  # [D, CS]
        tiled = (
            shard.reshape(ND, 128, NSUP, SUP)
            .transpose(2, 1, 0, 3)
            .reshape(NSUP * 128, ND * SUP)
        )
        in_maps.append(
            {
                "e": e,
                "wtgt": wtgt,
                "wt": np.ascontiguousarray(tiled),
            }
        )
    return in_maps


def kernel(embeddings, labels, weight, _trace=False):
    import time as _time

    nc = _get_nc()
    in_maps = make_in_maps(embeddings, labels, weight)
    last = None
    for attempt in range(4):
        try:
            res = run_bass_kernel_spmd(
                nc, in_maps, core_ids=list(range(NCORES)), trace=_trace
            )
            break
        except Exception as ex:  # transient axon/device hiccups: retry
            last = ex
            _time.sleep(5 * (attempt + 1))
    else:
        raise last
    out = np.asarray(res.results[0]["out"], dtype=np.float32).reshape(())
    if _trace:
        return out, res
    return out
